# revision 4
# baseline (speedup 1.0000x reference)
"""Distributed causal multi-head attention for TRN2 (8 NeuronCores).

Sharding: tensor-parallel over heads — core c owns heads {2c, 2c+1} for both
batches. QKV projections computed in transposed layout (feature on partitions,
tokens on free axis), attention computed as S.T = K @ Q.T per 128-key block
with softmax denominators obtained by augmenting V with a ones column, then an
AllToAll re-shards from head-parallel to token-parallel and each core applies
the softmax normalization and the output projection for its 512 flat tokens.
"""

import sys

sys.path.insert(0, "/opt/trn_rl_repo")

import numpy as np
import ml_dtypes

import concourse.bacc as bacc
import concourse.bass as bass
import concourse.mybir as mybir
import concourse.tile as tile
from concourse.bass_utils import run_bass_kernel_spmd

BF16 = mybir.dt.bfloat16
F32 = mybir.dt.float32
NPBF16 = ml_dtypes.bfloat16

B, T, C, H, D = 2, 2048, 1024, 16, 64
NCORES = 8
HPC = H // NCORES          # heads per core = 2
CP = HPC * D               # feature columns per core = 128
TF = B * T                 # flat tokens = 4096
TS = TF // NCORES          # output tokens per core = 512
NCB = C // 128             # feature blocks = 8
NTC = TF // 512            # 512-token chunks = 8
NQC = T // 512             # q-chunks per batch = 4
NKB = T // 128             # key blocks per batch = 16
SCALE = float(D) ** -0.5
MASKVAL = -30000.0


def build_nc():
    nc = bacc.Bacc("TRN2", target_bir_lowering=False, num_devices=NCORES)

    xT = nc.dram_tensor("xT", [C, TF], BF16, kind="ExternalInput")
    wqT = nc.dram_tensor("wqT", [C, CP], BF16, kind="ExternalInput")
    wkT = nc.dram_tensor("wkT", [C, CP], BF16, kind="ExternalInput")
    wvT = nc.dram_tensor("wvT", [C, CP], BF16, kind="ExternalInput")
    woT = nc.dram_tensor("woT", [C, C], BF16, kind="ExternalInput")
    # host-built constants
    sel = nc.dram_tensor("sel", [H, C], BF16, kind="ExternalInput")
    mtri = nc.dram_tensor("mtri", [128, 128], F32, kind="ExternalInput")
    ident = nc.dram_tensor("ident", [128, 64], BF16, kind="ExternalInput")
    out = nc.dram_tensor("out", [TS, C], F32, kind="ExternalOutput")

    with tile.TileContext(nc) as tc:
        with (
            tc.tile_pool(name="consts", bufs=1) as consts,
            tc.tile_pool(name="xp", bufs=1) as xp,
            tc.tile_pool(name="qkv", bufs=1) as qkv,
            tc.tile_pool(name="work", bufs=1) as work,
            tc.tile_pool(name="ps", bufs=1, space="PSUM") as psp,
            tc.tile_pool(name="dram", bufs=1, space="DRAM") as dram,
        ):
            # ---- load weights & constants ----
            wq_sb = consts.tile([128, NCB, CP], BF16)
            wk_sb = consts.tile([128, NCB, CP], BF16)
            wv_sb = consts.tile([128, NCB, CP], BF16)
            for cb in range(NCB):
                nc.sync.dma_start(wq_sb[:, cb, :], wqT[128 * cb : 128 * cb + 128, :])
                nc.sync.dma_start(wk_sb[:, cb, :], wkT[128 * cb : 128 * cb + 128, :])
                nc.sync.dma_start(wv_sb[:, cb, :], wvT[128 * cb : 128 * cb + 128, :])
            wo_sb = consts.tile([128, NCB, C], BF16)
            for cb in range(NCB):
                nc.sync.dma_start(wo_sb[:, cb, :], woT[128 * cb : 128 * cb + 128, :])
            sel_sb = consts.tile([H, C], BF16)
            nc.sync.dma_start(sel_sb[:], sel[:])
            mtri_sb = consts.tile([128, 128], F32)
            nc.sync.dma_start(mtri_sb[:], mtri[:])
            ident_sb = consts.tile([128, 64], BF16)
            nc.sync.dma_start(ident_sb[:], ident[:])

            # ---- load x (transposed) as 8 x [128, 4096], per 512-col subtiles ----
            x_sb = [xp.tile([128, TF], BF16, name=f"x_sb{cb}") for cb in range(NCB)]
            for cb in range(NCB):
                for tcn in range(NTC):
                    nc.sync.dma_start(
                        x_sb[cb][:, 512 * tcn : 512 * tcn + 512],
                        xT[128 * cb : 128 * cb + 128, 512 * tcn : 512 * tcn + 512],
                    )

            # ---- QKV projections (transposed layout [2*64, TF]) ----
            qT_sb = qkv.tile([128, TF], BF16)
            kT_sb = qkv.tile([128, TF], BF16)
            vT_sb = qkv.tile([128, TF], BF16)
            for w_sb, oT in ((wq_sb, qT_sb), (wk_sb, kT_sb), (wv_sb, vT_sb)):
                for tcn in range(NTC):
                    ps = psp.tile([128, 512], F32, tag="st", bufs=2, padded_shape=[128, 1024])
                    for cb in range(NCB):
                        nc.tensor.matmul(
                            ps[:],
                            lhsT=w_sb[:, cb, :],
                            rhs=x_sb[cb][:, 512 * tcn : 512 * tcn + 512],
                            start=(cb == 0),
                            stop=(cb == NCB - 1),
                        )
                    nc.vector.tensor_copy(oT[:, 512 * tcn : 512 * tcn + 512], ps[:])

            # ---- V into natural layout [keys, 64+1] via PE transposes ----
            # v_sb[pair][:, kb, 0:64] = V block; col 64 = ones (denominator trick)
            v_sb = [work.tile([128, NKB, 65], BF16, name=f"v_sb{p}") for p in range(4)]
            for pair in range(4):
                hh, b = pair // 2, pair % 2
                nc.vector.memset(v_sb[pair][:, :, 64:65], 1.0)
                for kb in range(NKB):
                    t0 = 2048 * b + 128 * kb
                    vt_ps = psp.tile([128, 64], BF16, tag="vt", bufs=2)
                    nc.tensor.transpose(
                        vt_ps[:],
                        vT_sb[64 * hh : 64 * hh + 64, t0 : t0 + 128],
                        ident_sb[64 * hh : 64 * hh + 64, :],
                    )
                    nc.vector.tensor_copy(v_sb[pair][:, kb, 0:64], vt_ps[:])

            # ---- A2A buffers: 8 chunks of [130, 512]: 128 yT rows + 2 denom rows
            a2a_in = dram.tile([NCORES, 130, 512], BF16)
            a2a_out = dram.tile([NCORES, 130, 512], BF16)

            # ---- attention per (head-half, batch) pair ----
            for pair in range(4):
                hh, b = pair // 2, pair % 2
                hs = slice(64 * hh, 64 * hh + 64)
                tb0 = 2048 * b
                for qc in range(NQC):
                    q_ap = qT_sb[hs, tb0 + 512 * qc : tb0 + 512 * qc + 512]
                    n_kb = 4 * qc + 4
                    ot = psp.tile([65, 512], F32, tag="ot", bufs=2)
                    for kbp in range(n_kb // 2):
                        st = psp.tile([128, 1024], F32, tag="st", bufs=2)
                        for h2 in range(2):
                            kb = 2 * kbp + h2
                            nc.tensor.matmul(
                                st[:, 512 * h2 : 512 * h2 + 512],
                                lhsT=kT_sb[hs, tb0 + 128 * kb : tb0 + 128 * kb + 128],
                                rhs=q_ap,
                                start=True,
                                stop=True,
                            )
                            off = 128 * kb - 512 * qc
                            if off >= 0:  # diagonal region
                                if off > 0:
                                    nc.vector.memset(
                                        st[:, 512 * h2 : 512 * h2 + off], MASKVAL
                                    )
                                d = slice(512 * h2 + off, 512 * h2 + off + 128)
                                nc.vector.tensor_add(st[:, d], st[:, d], mtri_sb[:])
                        pT = work.tile([128, 1024], BF16, tag="pT", bufs=3)
                        nc.scalar.activation(
                            pT[:], st[:], mybir.ActivationFunctionType.Exp, scale=SCALE
                        )
                        for h2 in range(2):
                            kb = 2 * kbp + h2
                            nc.tensor.matmul(
                                ot[:],
                                lhsT=v_sb[pair][:, kb, :],
                                rhs=pT[:, 512 * h2 : 512 * h2 + 512],
                                start=(kbp == 0 and h2 == 0),
                                stop=(kbp == n_kb // 2 - 1 and h2 == 1),
                            )
                    y_sb = work.tile([65, 512], BF16, tag="y", bufs=4)
                    nc.vector.tensor_copy(y_sb[:], ot[:])
                    s = 4 * b + qc
                    nc.sync.dma_start(
                        a2a_in[s, 64 * hh : 64 * hh + 64, :], y_sb[0:64, :]
                    )
                    nc.sync.dma_start(
                        a2a_in[s, 128 + hh : 129 + hh, :], y_sb[64:65, :]
                    )

            # ---- AllToAll: head-parallel -> token-parallel ----
            nc.gpsimd.collective_compute(
                "AllToAll",
                mybir.AluOpType.bypass,
                replica_groups=[list(range(NCORES))],
                ins=[a2a_in[:].opt()],
                outs=[a2a_out[:].opt()],
            )

            # ---- post-A2A: normalize + output projection ----
            y_loc = [work.tile([128, 512], BF16, name=f"y_loc{cb}") for cb in range(NCB)]
            for cb in range(NCB):
                nc.sync.dma_start(y_loc[cb][:], a2a_out[cb, 0:128, :])
            den_sb = work.tile([H, 512], BF16)
            nc.sync.dma_start(den_sb[:], a2a_out[:, 128:130, :])
            recip = work.tile([H, 512], F32)
            nc.vector.reciprocal(recip[:], den_sb[:])
            recip_bf = work.tile([H, 512], BF16)
            nc.vector.tensor_copy(recip_bf[:], recip[:])

            yn = [work.tile([128, 512], BF16, name=f"yn{cb}") for cb in range(NCB)]
            for cb in range(NCB):
                bc_ps = psp.tile([128, 512], F32, tag="st", bufs=2, padded_shape=[128, 1024])
                nc.tensor.matmul(
                    bc_ps[:],
                    lhsT=sel_sb[:, 128 * cb : 128 * cb + 128],
                    rhs=recip_bf[:],
                    start=True,
                    stop=True,
                )
                nc.vector.tensor_mul(yn[cb][:], y_loc[cb][:], bc_ps[:])

            for tb in range(4):
                for mh in range(2):
                    ps = psp.tile([128, 512], F32, tag="st", bufs=2, padded_shape=[128, 1024])
                    for cb in range(NCB):
                        nc.tensor.matmul(
                            ps[:],
                            lhsT=yn[cb][:, 128 * tb : 128 * tb + 128],
                            rhs=wo_sb[:, cb, 512 * mh : 512 * mh + 512],
                            start=(cb == 0),
                            stop=(cb == NCB - 1),
                        )
                    o_sb = work.tile([128, 512], F32, tag="osb", bufs=3)
                    nc.vector.tensor_copy(o_sb[:], ps[:])
                    nc.sync.dma_start(
                        out[128 * tb : 128 * tb + 128, 512 * mh : 512 * mh + 512],
                        o_sb[:],
                    )

    nc.compile()
    return nc


_NC = None


def _get_nc():
    global _NC
    if _NC is None:
        _NC = build_nc()
    return _NC


def _host_consts():
    sel = np.zeros((H, C), dtype=np.float32)
    for h in range(H):
        sel[h, 64 * h : 64 * h + 64] = 1.0
    idx = np.arange(128)
    mtri = np.where(idx[None, :] >= idx[:, None], 0.0, MASKVAL).astype(np.float32)
    ident = np.concatenate([np.eye(64, dtype=np.float32)] * 2, axis=0)
    return sel.astype(NPBF16), mtri, ident.astype(NPBF16)


def kernel(x, mask, Wq, Wk, Wv, Wo):
    del mask  # causal mask is hardcoded in the device kernel
    x = np.asarray(x, dtype=np.float32)
    Wq = np.asarray(Wq, dtype=np.float32)
    Wk = np.asarray(Wk, dtype=np.float32)
    Wv = np.asarray(Wv, dtype=np.float32)
    Wo = np.asarray(Wo, dtype=np.float32)

    nc = _get_nc()
    xT = np.ascontiguousarray(x.reshape(TF, C).T).astype(NPBF16)
    woT = np.ascontiguousarray(Wo.T).astype(NPBF16)
    sel, mtri, ident = _host_consts()

    in_maps = []
    for c in range(NCORES):
        rows = slice(CP * c, CP * c + CP)
        in_maps.append(
            {
                "xT": xT,
                "wqT": np.ascontiguousarray(Wq[rows].T).astype(NPBF16),
                "wkT": np.ascontiguousarray(Wk[rows].T).astype(NPBF16),
                "wvT": np.ascontiguousarray(Wv[rows].T).astype(NPBF16),
                "woT": woT,
                "sel": sel,
                "mtri": mtri,
                "ident": ident,
            }
        )

    res = run_bass_kernel_spmd(nc, in_maps, core_ids=list(range(NCORES)))
    out = np.concatenate([res.results[c]["out"] for c in range(NCORES)], axis=0)
    return out.reshape(B, T, C).astype(np.float32)


# revision 6
# speedup vs baseline: 1.2062x; 1.2062x over previous
"""Distributed causal multi-head attention for TRN2 (8 NeuronCores).

Sharding: tensor-parallel over heads — core c owns heads {2c, 2c+1} for both
batches. QKV projections computed in transposed layout (feature on partitions,
tokens on free axis), attention computed as S.T = K @ Q.T per 128-key block
with softmax denominators obtained by augmenting V with a ones column. Two
batch-split AllToAlls re-shard from head-parallel to token-parallel; each core
then applies softmax normalization and the output projection for its tokens.

Emission order interleaves batch-1 QKV into batch-0 attention (and batch-0
post-processing into batch-1 attention) to keep the PE stream dense.
"""

import sys

sys.path.insert(0, "/opt/trn_rl_repo")

import numpy as np
import ml_dtypes

import concourse.bacc as bacc
import concourse.bass as bass
import concourse.mybir as mybir
import concourse.tile as tile
from concourse.bass_utils import run_bass_kernel_spmd

BF16 = mybir.dt.bfloat16
F32 = mybir.dt.float32
NPBF16 = ml_dtypes.bfloat16

B, T, C, H, D = 2, 2048, 1024, 16, 64
NCORES = 8
HPC = H // NCORES          # heads per core = 2
CP = HPC * D               # feature columns per core = 128
TF = B * T                 # flat tokens = 4096
TS = TF // NCORES          # output tokens per core = 512 (256 per batch)
TSB = TS // B              # 256
NCB = C // 128             # feature blocks = 8
NQC = T // 512             # q-chunks per batch = 4
NKB = T // 128             # key blocks per batch = 16
SCALE = float(D) ** -0.5
MASKVAL = -30000.0


def build_nc():
    nc = bacc.Bacc("TRN2", target_bir_lowering=False, num_devices=NCORES)

    xT = nc.dram_tensor("xT", [C, TF], BF16, kind="ExternalInput")
    wqT = nc.dram_tensor("wqT", [C, CP], BF16, kind="ExternalInput")
    wkT = nc.dram_tensor("wkT", [C, CP], BF16, kind="ExternalInput")
    wvT = nc.dram_tensor("wvT", [C, CP], BF16, kind="ExternalInput")
    woT = nc.dram_tensor("woT", [C, C], BF16, kind="ExternalInput")
    sel = nc.dram_tensor("sel", [H, C], BF16, kind="ExternalInput")
    mtri = nc.dram_tensor("mtri", [128, 128], F32, kind="ExternalInput")
    ident = nc.dram_tensor("ident", [128, 64], BF16, kind="ExternalInput")
    out = nc.dram_tensor("out", [TS, C], F32, kind="ExternalOutput")

    with tile.TileContext(nc) as tc:
        with (
            tc.tile_pool(name="consts", bufs=1) as consts,
            tc.tile_pool(name="xp", bufs=1) as xp,
            tc.tile_pool(name="qkv", bufs=1) as qkv,
            tc.tile_pool(name="work", bufs=1) as work,
            tc.tile_pool(name="ps", bufs=1, space="PSUM") as psp,
            tc.tile_pool(name="dram", bufs=1, space="DRAM") as dram,
        ):
            dmae = [nc.sync, nc.gpsimd, nc.sync, nc.scalar]

            # ---- weights & constants (single strided DMAs) ----
            wq_sb = consts.tile([128, NCB, CP], BF16)
            wk_sb = consts.tile([128, NCB, CP], BF16)
            wv_sb = consts.tile([128, NCB, CP], BF16)
            for i, (w_sb, wdr) in enumerate(
                ((wq_sb, wqT), (wk_sb, wkT), (wv_sb, wvT))
            ):
                src = bass.AP(wdr, 0, [[CP, 128], [128 * CP, NCB], [1, CP]])
                dmae[i % 4].dma_start(w_sb[:], src)
            sel_sb = consts.tile([H, C], BF16)
            nc.scalar.dma_start(sel_sb[:], sel[:])
            mtri_sb = consts.tile([128, 128], F32)
            nc.scalar.dma_start(mtri_sb[:], mtri[:])
            ident_sb = consts.tile([128, 64], BF16)
            nc.gpsimd.dma_start(ident_sb[:], ident[:])

            # ---- x loads: chunk-major so early QKV can start immediately ----
            x_sb = [xp.tile([128, TF], BF16, name=f"x_sb{cb}") for cb in range(NCB)]
            for tcn in range(8):
                for cb in range(NCB):
                    dmae[(tcn * NCB + cb) % 4].dma_start(
                        x_sb[cb][:, 512 * tcn : 512 * tcn + 512],
                        xT[128 * cb : 128 * cb + 128, 512 * tcn : 512 * tcn + 512],
                    )

            # wo late: not needed until output projection
            wo_sb = consts.tile([128, NCB, C], BF16)
            wo_src = bass.AP(woT, 0, [[C, 128], [128 * C, NCB], [1, C]])
            nc.sync.dma_start(wo_sb[:], wo_src)

            qT_sb = qkv.tile([128, TF], BF16)
            kT_sb = qkv.tile([128, TF], BF16)
            vT_sb = qkv.tile([128, TF], BF16)
            projs = ((wq_sb, qT_sb), (wk_sb, kT_sb), (wv_sb, vT_sb))

            v_sb = [work.tile([128, NKB, 65], BF16, name=f"v_sb{p}") for p in range(4)]

            a2a_in = [
                dram.tile([NCORES, 130, TSB], BF16, name=f"a2a_in{b}")
                for b in range(B)
            ]
            a2a_out = [
                dram.tile([NCORES, 130, TSB], BF16, name=f"a2a_out{b}")
                for b in range(B)
            ]

            # ---------- emission units ----------
            def qkv_unit(tcn, pi):
                w_sb, oT = projs[pi]
                ps = psp.tile(
                    [128, 512], F32, tag="st", bufs=2,
                    padded_shape=[128, 1024], name="ps_proj",
                )
                for cb in range(NCB):
                    nc.tensor.matmul(
                        ps[:],
                        lhsT=w_sb[:, cb, :],
                        rhs=x_sb[cb][:, 512 * tcn : 512 * tcn + 512],
                        start=(cb == 0),
                        stop=(cb == NCB - 1),
                    )
                nc.vector.tensor_copy(oT[:, 512 * tcn : 512 * tcn + 512], ps[:])

            def vt_unit(pair, kb):
                hh, b = pair % 2, pair // 2
                if kb == 0:
                    nc.vector.memset(v_sb[pair][:, :, 64:65], 1.0)
                t0 = 2048 * b + 128 * kb
                vt_ps = psp.tile([128, 64], BF16, tag="vt", bufs=2, name="vt_ps")
                nc.tensor.transpose(
                    vt_ps[:],
                    vT_sb[64 * hh : 64 * hh + 64, t0 : t0 + 128],
                    ident_sb[64 * hh : 64 * hh + 64, :],
                )
                nc.vector.tensor_copy(v_sb[pair][:, kb, 0:64], vt_ps[:])

            # attention state per (pair, qc), lives across kbp units
            attn_ot = {}

            def attn_unit(pair, qc, kbp):
                hh, b = pair % 2, pair // 2
                hs = slice(64 * hh, 64 * hh + 64)
                tb0 = 2048 * b
                q0 = tb0 + 512 * qc
                if kbp == 0:
                    attn_ot[(pair, qc)] = psp.tile(
                        [65, 512], F32, tag="ot", bufs=2, name="ot_ps"
                    )
                ot = attn_ot[(pair, qc)]
                n_kb = 4 * qc + 4
                st = psp.tile([128, 1024], F32, tag="st", bufs=2, name="st_ps")
                offs = []
                for h2 in range(2):
                    kb = 2 * kbp + h2
                    off = max(0, 128 * kb - 512 * qc)
                    offs.append(off)
                    nc.tensor.matmul(
                        st[:, 512 * h2 + off : 512 * h2 + 512],
                        lhsT=kT_sb[hs, tb0 + 128 * kb : tb0 + 128 * kb + 128],
                        rhs=qT_sb[hs, q0 + off : q0 + 512],
                        start=True,
                        stop=True,
                    )
                    if 128 * kb >= 512 * qc:  # diagonal block: triangular mask
                        dd = slice(512 * h2 + off, 512 * h2 + off + 128)
                        nc.vector.tensor_add(st[:, dd], st[:, dd], mtri_sb[:])
                pT = work.tile([128, 1024], BF16, tag="pT", bufs=3, name="pT")
                o0 = offs[0]
                nc.scalar.activation(
                    pT[:, o0:1024],
                    st[:, o0:1024],
                    mybir.ActivationFunctionType.Exp,
                    scale=SCALE,
                )
                for h2 in range(2):
                    kb = 2 * kbp + h2
                    off = offs[h2]
                    nc.tensor.matmul(
                        ot[:, off:512],
                        lhsT=v_sb[pair][:, kb, :],
                        rhs=pT[:, 512 * h2 + off : 512 * h2 + 512],
                        start=(kb == 0),
                        stop=(kb == n_kb - 1),
                    )

            def evict_unit(pair, qc, eng):
                hh, b = pair % 2, pair // 2
                ot = attn_ot.pop((pair, qc))
                y_sb = work.tile([65, 512], BF16, tag="y", bufs=4, name="y_sb")
                nc.vector.tensor_copy(y_sb[:], ot[:])
                for s2 in range(2):
                    s = 2 * qc + s2
                    eng.dma_start(
                        a2a_in[b][s, 64 * hh : 64 * hh + 64, :],
                        y_sb[0:64, 256 * s2 : 256 * s2 + 256],
                    )
                    eng.dma_start(
                        a2a_in[b][s, 128 + hh : 129 + hh, :],
                        y_sb[64:65, 256 * s2 : 256 * s2 + 256],
                    )

            def attn_units_for_batch(b):
                units = []
                for qc in range(NQC):
                    for hh in range(2):
                        pair = 2 * b + hh
                        for kbp in range((4 * qc + 4) // 2):
                            units.append(("a", pair, qc, kbp))
                        units.append(("e", pair, qc))
                return units

            def run_unit(u, i):
                if u[0] == "a":
                    attn_unit(u[1], u[2], u[3])
                elif u[0] == "e":
                    evict_unit(u[1], u[2], dmae[i % 4])
                elif u[0] == "q":
                    qkv_unit(u[1], u[2])
                elif u[0] == "v":
                    vt_unit(u[1], u[2])

            def interleave(main, feed):
                fi = 0
                ratio = len(feed) / max(len(main), 1)
                for i, u in enumerate(main):
                    run_unit(u, i)
                    want = int(round((i + 1) * ratio))
                    while fi < want:
                        run_unit(feed[fi], fi)
                        fi += 1
                while fi < len(feed):
                    run_unit(feed[fi], fi)
                    fi += 1

            # ---------- phase A: QKV b0 + V-transposes b0 ----------
            for tcn in range(4):
                for pi in range(3):
                    qkv_unit(tcn, pi)
            for hh in range(2):
                for kb in range(NKB):
                    vt_unit(hh, kb)  # pairs 0,1 are batch 0

            # ---------- phase B: attention b0 with QKV/vt b1 interleaved ----
            feed_b1 = [("q", tcn, pi) for tcn in range(4, 8) for pi in range(3)]
            feed_b1 += [("v", 2 + hh, kb) for hh in range(2) for kb in range(NKB)]
            interleave(attn_units_for_batch(0), feed_b1)

            # ---------- A2A #1 (batch 0) ----------
            nc.gpsimd.collective_compute(
                "AllToAll",
                mybir.AluOpType.bypass,
                replica_groups=[list(range(NCORES))],
                ins=[a2a_in[0][:].opt()],
                outs=[a2a_out[0][:].opt()],
            )

            # ---------- post-processing units for one batch ----------
            def post_batch(b):
                y_loc = [
                    work.tile([128, TSB], BF16, tag=f"yloc{cb}", name=f"y_loc{cb}")
                    for cb in range(NCB)
                ]
                for cb in range(NCB):
                    dmae[cb % 4].dma_start(y_loc[cb][:], a2a_out[b][cb, 0:128, :])
                den_sb = work.tile([H, TSB], BF16, tag="den", name="den_sb")
                nc.sync.dma_start(den_sb[:], a2a_out[b][:, 128:130, :])
                recip = work.tile([H, TSB], F32, tag="recip", name="recip")
                nc.vector.reciprocal(recip[:], den_sb[:])
                recip_bf = work.tile([H, TSB], BF16, tag="recipbf", name="recip_bf")
                nc.vector.tensor_copy(recip_bf[:], recip[:])
                yn = [
                    work.tile([128, TSB], BF16, tag=f"yn{cb}", name=f"yn{cb}")
                    for cb in range(NCB)
                ]

                def norm_unit(cb):
                    bc_ps = psp.tile([128, TSB], F32, tag="vt", bufs=2, name="bc_ps")
                    nc.tensor.matmul(
                        bc_ps[:],
                        lhsT=sel_sb[:, 128 * cb : 128 * cb + 128],
                        rhs=recip_bf[:],
                        start=True,
                        stop=True,
                    )
                    nc.vector.tensor_mul(yn[cb][:], y_loc[cb][:], bc_ps[:])

                def proj_unit(tb, mh):
                    ps = psp.tile(
                        [128, 512], F32, tag="st", bufs=2,
                        padded_shape=[128, 1024], name="ps_op",
                    )
                    for cb in range(NCB):
                        nc.tensor.matmul(
                            ps[:],
                            lhsT=yn[cb][:, 128 * tb : 128 * tb + 128],
                            rhs=wo_sb[:, cb, 512 * mh : 512 * mh + 512],
                            start=(cb == 0),
                            stop=(cb == NCB - 1),
                        )
                    o_sb = work.tile([128, 512], F32, tag="osb", bufs=3, name="o_sb")
                    nc.vector.tensor_copy(o_sb[:], ps[:])
                    nc.sync.dma_start(
                        out[
                            TSB * b + 128 * tb : TSB * b + 128 * tb + 128,
                            512 * mh : 512 * mh + 512,
                        ],
                        o_sb[:],
                    )

                units = [(norm_unit, (cb,)) for cb in range(NCB)]
                units += [(proj_unit, (tb, mh)) for tb in range(2) for mh in range(2)]
                return units

            # ---------- phase D: attention b1 with post-b0 interleaved -----
            post0 = post_batch(0)
            main_b1 = attn_units_for_batch(1)
            fi = 0
            ratio = len(post0) / max(len(main_b1) - 8, 1)
            for i, u in enumerate(main_b1):
                run_unit(u, i)
                if i >= 8:
                    want = int(round((i - 7) * ratio))
                    while fi < min(want, len(post0)):
                        f, args = post0[fi]
                        f(*args)
                        fi += 1
            while fi < len(post0):
                f, args = post0[fi]
                f(*args)
                fi += 1

            # ---------- A2A #2 (batch 1) + its post ----------
            nc.gpsimd.collective_compute(
                "AllToAll",
                mybir.AluOpType.bypass,
                replica_groups=[list(range(NCORES))],
                ins=[a2a_in[1][:].opt()],
                outs=[a2a_out[1][:].opt()],
            )
            for f, args in post_batch(1):
                f(*args)

    nc.compile()
    return nc


_NC = None


def _get_nc():
    global _NC
    if _NC is None:
        _NC = build_nc()
    return _NC


def _host_consts():
    sel = np.zeros((H, C), dtype=np.float32)
    for h in range(H):
        sel[h, 64 * h : 64 * h + 64] = 1.0
    idx = np.arange(128)
    mtri = np.where(idx[None, :] >= idx[:, None], 0.0, MASKVAL).astype(np.float32)
    ident = np.concatenate([np.eye(64, dtype=np.float32)] * 2, axis=0)
    return sel.astype(NPBF16), mtri, ident.astype(NPBF16)


def _make_in_maps(x, Wq, Wk, Wv, Wo):
    xT = np.ascontiguousarray(x.reshape(TF, C).T).astype(NPBF16)
    woT = np.ascontiguousarray(Wo.T).astype(NPBF16)
    sel, mtri, ident = _host_consts()
    in_maps = []
    for c in range(NCORES):
        rows = slice(CP * c, CP * c + CP)
        in_maps.append(
            {
                "xT": xT,
                "wqT": np.ascontiguousarray(Wq[rows].T).astype(NPBF16),
                "wkT": np.ascontiguousarray(Wk[rows].T).astype(NPBF16),
                "wvT": np.ascontiguousarray(Wv[rows].T).astype(NPBF16),
                "woT": woT,
                "sel": sel,
                "mtri": mtri,
                "ident": ident,
            }
        )
    return in_maps


def _assemble(results):
    # core c's out rows: [0:256] = batch0 tokens [256c:256c+256),
    #                    [256:512] = batch1 tokens [256c:256c+256)
    full = np.zeros((B, T, C), dtype=np.float32)
    for c in range(NCORES):
        o = results[c]["out"]
        full[0, TSB * c : TSB * c + TSB] = o[0:TSB]
        full[1, TSB * c : TSB * c + TSB] = o[TSB : 2 * TSB]
    return full


def kernel(x, mask, Wq, Wk, Wv, Wo):
    del mask  # causal mask is hardcoded in the device kernel
    in_maps = _make_in_maps(
        np.asarray(x, dtype=np.float32),
        np.asarray(Wq, dtype=np.float32),
        np.asarray(Wk, dtype=np.float32),
        np.asarray(Wv, dtype=np.float32),
        np.asarray(Wo, dtype=np.float32),
    )
    nc = _get_nc()
    res = run_bass_kernel_spmd(nc, in_maps, core_ids=list(range(NCORES)))
    return _assemble(res.results)
